# revision 4
# baseline (speedup 1.0000x reference)
"""Causal multi-head attention block (QKV proj + attention + out proj) on 8
TRN2 NeuronCores.

Tensor-parallel sharding: core c handles batch b = c//4 and head group
g = c%4 (heads 4g..4g+3).  QKV weights are column-sharded, w_proj is
row-sharded; each core emits a partial output that the host sums per batch
(the row-parallel "all-reduce" happens on host since full outputs are
gathered anyway).

Shapes (hardcoded): B=2, T=2048, D=1024, H=16, dh=64.
Returns (a [B,T,D], present [2,B,H,T,dh]) matching the reference.

All matmuls run in float32r (TF32-like, full PE rate, ~1e-4 rel err).
Scores are computed transposed (S^T[k,q] = k @ q^T) so the attention
weights feed straight into A^T = (v | 1)^T-style matmuls without any
transposes: the (v | ones) stationary trick makes row 64 of the PV psum
the softmax denominator for free.  Causality is exploited by skipping
fully-masked k-tiles and masking the 4 diagonal-crossing tiles.
"""
import numpy as np

import concourse.bass as bass
import concourse.bacc as bacc
import concourse.tile as tile
import concourse.mybir as mybir
from concourse.bass_utils import run_bass_kernel_spmd

B, T, D = 2, 2048, 1024
H = 16
DH = 64
HPC = 4            # heads per core
DL = HPC * DH      # 256 local head dims per core
KD = D // 128      # 8 contraction tiles over D
NT = T // 128      # 16 tiles over T (key positions / output rows)
NQ = T // 512      # 4 query tiles of 512
F32 = mybir.dt.float32
F32R = mybir.dt.float32r

_CACHE = {}


def build():
    nc = bacc.Bacc(
        "TRN2",
        target_bir_lowering=False,
        debug=False,
        enable_asserts=True,
        num_devices=8,
    )
    xT = nc.declare_dram_parameter("xT", (D, T), F32, isOutput=False)
    w_qk = nc.declare_dram_parameter("w_qk", (D, 2 * DL), F32, isOutput=False)
    w_v = nc.declare_dram_parameter("w_v", (D, DL), F32, isOutput=False)
    b_qk = nc.declare_dram_parameter("b_qk", (2 * DL,), F32, isOutput=False)
    b_v = nc.declare_dram_parameter("b_v", (DL,), F32, isOutput=False)
    w_p = nc.declare_dram_parameter("w_p", (DL, D), F32, isOutput=False)
    o_part = nc.declare_dram_parameter("o_part", (T, D), F32, isOutput=True)
    kT_out = nc.declare_dram_parameter("kT_out", (DL, T), F32, isOutput=True)
    v_out = nc.declare_dram_parameter("v_out", (T, DL), F32, isOutput=True)

    with tile.TileContext(nc) as tc:
        with (
            tc.tile_pool(name="big", bufs=1) as big,
            tc.tile_pool(name="wqk", bufs=4) as wqkp,
            tc.tile_pool(name="exp", bufs=6) as expp,
            tc.tile_pool(name="nrm", bufs=4) as nrmp,
            tc.tile_pool(name="ost", bufs=2) as ostp,
            tc.tile_pool(name="ps", bufs=8, space="PSUM") as ps,
        ):
            # ---- resident inputs -------------------------------------------
            xT_sb = big.tile([128, KD, T], F32R, tag="xT")
            for kt in range(KD):
                nc.gpsimd.dma_start(
                    out=xT_sb[:, kt, :], in_=xT[128 * kt : 128 * (kt + 1), :]
                )
            w_v_sb = big.tile([128, KD, DL], F32R, tag="wv")
            nc.gpsimd.dma_start(
                out=w_v_sb, in_=w_v.rearrange("(kt p) n -> p kt n", p=128)
            )
            w_p_sb = big.tile([128, 2, D], F32R, tag="wp")
            nc.gpsimd.dma_start(
                out=w_p_sb, in_=w_p.rearrange("(kt p) n -> p kt n", p=128)
            )
            b_qk_sb = big.tile([128, 4], F32, tag="bqk")
            nc.sync.dma_start(
                out=b_qk_sb, in_=b_qk.rearrange("(m p) -> p m", p=128)
            )
            ones4 = big.tile([128, 4], F32, tag="ones4")
            nc.vector.memset(ones4, 1.0)
            bv_ap = b_v[:]
            bv_bc = big.tile([128, DL], F32, tag="bv")
            nc.sync.dma_start(
                out=bv_bc,
                in_=bass.AP(tensor=bv_ap.tensor, offset=0, ap=[[0, 128], [1, DL]]),
            )
            # ---- diagonal causal masks (k<=q), offsets d = 128*i -----------
            mask_f = big.tile([128, 512], F32, tag="maskf")
            masks = big.tile([128, 4, 512], F32R, tag="masks")
            for i in range(4):
                nc.gpsimd.memset(mask_f[:, :], 1.0)
                nc.gpsimd.affine_select(
                    out=mask_f[:, :],
                    in_=mask_f[:, :],
                    compare_op=mybir.AluOpType.is_ge,
                    fill=0.0,
                    base=-128 * i,
                    pattern=[[1, 512]],
                    channel_multiplier=-1,
                )
                nc.vector.tensor_copy(masks[:, i, :], mask_f[:, :])

            # ---- QKV projection --------------------------------------------
            # q/k transposed: qkT[m,t] tiles, m in {q01,q23,k01,k23}
            qkT_sb = big.tile([128, 4, T], F32R, tag="qkT")
            for m in (0, 2, 1, 3):
                psums = [ps.tile([128, 512], F32, tag="ps", name=f"ps_qk{m}_{i}") for i in range(NQ)]
                for kt in range(KD):
                    wt = wqkp.tile([128, 128], F32R, tag="wqk")
                    nc.gpsimd.dma_start(
                        out=wt,
                        in_=w_qk[128 * kt : 128 * (kt + 1), 128 * m : 128 * (m + 1)],
                    )
                    for n in range(NQ):
                        nc.tensor.matmul(
                            psums[n],
                            wt,
                            xT_sb[:, kt, 512 * n : 512 * (n + 1)],
                            start=(kt == 0),
                            stop=(kt == KD - 1),
                        )
                for n in range(NQ):
                    nc.scalar.activation(
                        qkT_sb[:, m, 512 * n : 512 * (n + 1)],
                        psums[n],
                        mybir.ActivationFunctionType.Identity,
                        bias=b_qk_sb[:, m : m + 1],
                    )
                if m >= 2:
                    nc.sync.dma_start(
                        out=kT_out[128 * (m - 2) : 128 * (m - 1), :],
                        in_=qkT_sb[:, m, :].bitcast(F32),
                    )
                # v projection in natural [t, dh] layout, with a ones column
                # per head appended (row 64 of PV psum = softmax denominator)
                if m == 2:
                    v_sb = big.tile([128, NT, HPC * 65], F32R, tag="v")
                    for t in range(NT):
                        pv = ps.tile([128, 512], F32, tag="ps")
                        for kt in range(KD):
                            nc.tensor.matmul(
                                pv[:, :DL],
                                xT_sb[:, kt, 128 * t : 128 * (t + 1)],
                                w_v_sb[:, kt, :],
                                start=(kt == 0),
                                stop=(kt == KD - 1),
                            )
                        vslice = v_sb[:, t, :].rearrange("p (h c) -> p h c", h=HPC)
                        nc.vector.tensor_add(
                            vslice[:, :, 0:64],
                            pv[:, :DL].rearrange("p (h c) -> p h c", c=64),
                            bv_bc.rearrange("p (h c) -> p h c", c=64),
                        )
                        nc.vector.tensor_copy(
                            vslice[:, :, 64:65], ones4.rearrange("p (h o) -> p h o", o=1)
                        )
                        nc.sync.dma_start(
                            out=v_out[128 * t : 128 * (t + 1), :].rearrange(
                                "p (h c) -> p h c", h=HPC
                            ),
                            in_=vslice[:, :, 0:64].bitcast(F32),
                        )

            # ---- attention: S^T scores, exp, (v|1)-PV, normalize -----------
            AT_sb = big.tile([128, 2, T], F32R, tag="AT")
            stages = [(hp, qj) for qj in range(NQ) for hp in range(2)]
            LOOKAHEAD = 2

            def emit_pv(st, kt, exps, pvs):
                hp, qj = st
                nk = 4 * qj + 4
                for par in range(2):
                    h = 2 * hp + par
                    nc.tensor.matmul(
                        pvs[par][:65, :],
                        v_sb[:, kt, 65 * h : 65 * (h + 1)],
                        exps[kt][par],
                        start=(kt == 0),
                        stop=(kt == nk - 1),
                    )

            def emit_scores(st, kt, exps):
                hp, qj = st
                pair = []
                for par in range(2):
                    p0 = 64 * par
                    pss = ps.tile([128, 512], F32, tag="ps")
                    nc.tensor.matmul(
                        pss,
                        qkT_sb[p0 : p0 + 64, 2 + hp, 128 * kt : 128 * (kt + 1)],
                        qkT_sb[p0 : p0 + 64, hp, 512 * qj : 512 * (qj + 1)],
                        start=True,
                        stop=True,
                    )
                    et = expp.tile([128, 512], F32R, tag="exp")
                    nc.scalar.activation(
                        et, pss, mybir.ActivationFunctionType.Exp, scale=0.25
                    )
                    if kt >= 4 * qj:
                        nc.vector.tensor_mul(et, et, masks[:, kt - 4 * qj, :])
                    pair.append(et)
                exps[kt] = pair

            def emit_norm(st, pvs):
                hp, qj = st
                for par in range(2):
                    rc = nrmp.tile([1, 512], F32, tag="recip")
                    nc.vector.reciprocal(rc, pvs[par][64:65, :])
                    bc = nrmp.tile([64, 512], F32, tag="bcast")
                    nc.gpsimd.partition_broadcast(bc, rc[0:1, :])
                    nc.vector.tensor_mul(
                        AT_sb[64 * par : 64 * (par + 1), hp, 512 * qj : 512 * (qj + 1)],
                        pvs[par][0:64, :],
                        bc,
                    )

            for st in stages:
                hp, qj = st
                nk = 4 * qj + 4
                exps = {}
                pvs = [
                    ps.tile([128, 512], F32, tag="ps", name=f"ps_pv{hp}_{qj}_{i}")
                    for i in range(2)
                ]
                for kt in range(nk):
                    emit_scores(st, kt, exps)
                    if kt >= LOOKAHEAD:
                        emit_pv(st, kt - LOOKAHEAD, exps, pvs)
                        del exps[kt - LOOKAHEAD]
                for kt in range(max(0, nk - LOOKAHEAD), nk):
                    emit_pv(st, kt, exps, pvs)
                emit_norm(st, pvs)

            # ---- output projection -----------------------------------------
            for t in range(NT):
                po = [ps.tile([128, 512], F32, tag="ps", name=f"ps_o{t}_{i}") for i in range(2)]
                for dd in range(2):
                    for kt in range(2):
                        nc.tensor.matmul(
                            po[dd],
                            AT_sb[:, kt, 128 * t : 128 * (t + 1)],
                            w_p_sb[:, kt, 512 * dd : 512 * (dd + 1)],
                            start=(kt == 0),
                            stop=(kt == 1),
                        )
                o_sb = ostp.tile([128, D], F32, tag="o")
                for dd in range(2):
                    nc.vector.tensor_copy(o_sb[:, 512 * dd : 512 * (dd + 1)], po[dd])
                nc.sync.dma_start(
                    out=o_part[128 * t : 128 * (t + 1), :], in_=o_sb
                )
    nc.finalize()
    return nc


def _shard_inputs(x, w_attn, b_attn, w_proj):
    in_maps = []
    for c in range(8):
        bi, g = divmod(c, 4)
        cq = slice(DL * g, DL * (g + 1))
        ck = slice(D + DL * g, D + DL * (g + 1))
        cv = slice(2 * D + DL * g, 2 * D + DL * (g + 1))
        in_maps.append(
            {
                "xT": np.ascontiguousarray(x[bi].T),
                "w_qk": np.ascontiguousarray(
                    np.concatenate([w_attn[:, cq], w_attn[:, ck]], axis=1)
                ),
                "w_v": np.ascontiguousarray(w_attn[:, cv]),
                "b_qk": np.ascontiguousarray(
                    np.concatenate([b_attn[cq], b_attn[ck]])
                ),
                "b_v": np.ascontiguousarray(b_attn[cv]),
                "w_p": np.ascontiguousarray(w_proj[DL * g : DL * (g + 1), :]),
            }
        )
    return in_maps


def _gather(results, b_proj):
    a = np.zeros((B, T, D), np.float32)
    present = np.zeros((2, B, H, T, DH), np.float32)
    for c in range(8):
        bi, g = divmod(c, 4)
        r = results[c]
        a[bi] += r["o_part"]
        present[0, bi, HPC * g : HPC * (g + 1)] = (
            r["kT_out"].reshape(HPC, DH, T).transpose(0, 2, 1)
        )
        present[1, bi, HPC * g : HPC * (g + 1)] = (
            r["v_out"].reshape(T, HPC, DH).transpose(1, 0, 2)
        )
    a += b_proj.astype(np.float32)
    return a, present


def run(x, w_attn, b_attn, w_proj, b_proj, trace=False):
    if "nc" not in _CACHE:
        _CACHE["nc"] = build()
    in_maps = _shard_inputs(
        np.asarray(x, np.float32),
        np.asarray(w_attn, np.float32),
        np.asarray(b_attn, np.float32),
        np.asarray(w_proj, np.float32),
    )
    res = run_bass_kernel_spmd(
        _CACHE["nc"], in_maps, core_ids=list(range(8)), trace=trace
    )
    a, present = _gather(res.results, np.asarray(b_proj, np.float32))
    return (a, present), res


def kernel(x, w_attn, b_attn, w_proj, b_proj):
    (a, present), _ = run(x, w_attn, b_attn, w_proj, b_proj, trace=False)
    return a, present


# revision 6
# speedup vs baseline: 1.2394x; 1.2394x over previous
"""Causal multi-head attention block (QKV proj + attention + out proj) on 8
TRN2 NeuronCores.

Tensor-parallel sharding: core c handles batch b = c//4 and head group
g = c%4 (heads 4g..4g+3).  QKV weights are column-sharded, w_proj is
row-sharded; each core emits a partial output that the host sums per batch
(the row-parallel "all-reduce" happens on host since full outputs are
gathered anyway).

Shapes (hardcoded): B=2, T=2048, D=1024, H=16, dh=64.
Returns (a [B,T,D], present [2,B,H,T,dh]) matching the reference.

All matmuls run in float32r (TF32-like, full PE rate, ~1e-4 rel err).
Scores are computed transposed (S^T[k,q] = k @ q^T) so the attention
weights feed straight into the PV matmuls without transposing P: the
(v | ones) stationary trick makes row 64 of the PV psum the softmax
denominator for free.  Causality skips fully-masked k-tiles; the 4
diagonal-crossing tile shapes are masked with precomputed 0/1 tiles.
Head pairs share a 2-bank score psum so one Exp covers both heads.
"""
import numpy as np

import concourse.bass as bass
import concourse.bacc as bacc
import concourse.tile as tile
import concourse.mybir as mybir
from concourse.bass_utils import run_bass_kernel_spmd

B, T, D = 2, 2048, 1024
H = 16
DH = 64
HPC = 4            # heads per core
DL = HPC * DH      # 256 local head dims per core
KD = D // 128      # 8 contraction tiles over D
NT = T // 128      # 16 tiles over T (key positions / output rows)
NQ = T // 512      # 4 query tiles of 512
F32 = mybir.dt.float32
F32R = mybir.dt.float32r
LOOK = 2           # PV trails scores by this many k-tiles

_CACHE = {}


def build():
    nc = bacc.Bacc(
        "TRN2",
        target_bir_lowering=False,
        debug=False,
        enable_asserts=True,
        num_devices=8,
    )
    xT = nc.declare_dram_parameter("xT", (D, T), F32R, isOutput=False)
    w_qk = nc.declare_dram_parameter("w_qk", (D, 2 * DL), F32R, isOutput=False)
    w_v = nc.declare_dram_parameter("w_v", (D, DL), F32R, isOutput=False)
    b_qk = nc.declare_dram_parameter("b_qk", (2 * DL,), F32, isOutput=False)
    b_v = nc.declare_dram_parameter("b_v", (DL,), F32, isOutput=False)
    w_p = nc.declare_dram_parameter("w_p", (DL, D), F32R, isOutput=False)
    o_part = nc.declare_dram_parameter("o_part", (T, D), F32, isOutput=True)
    kT_out = nc.declare_dram_parameter("kT_out", (DL, T), F32, isOutput=True)
    v_out = nc.declare_dram_parameter("v_out", (T, DL), F32, isOutput=True)

    with tile.TileContext(nc) as tc:
        with (
            tc.tile_pool(name="big", bufs=1) as big,
            tc.tile_pool(name="wqk", bufs=4) as wqkp,
            tc.tile_pool(name="exp", bufs=4) as expp,
            tc.tile_pool(name="nrm", bufs=3) as nrmp,
            tc.tile_pool(name="ost", bufs=2) as ostp,
            tc.tile_pool(name="ps", bufs=1, space="PSUM") as ps,
        ):
            # ---- resident inputs (all via fast HW-DGE DMA) -----------------
            xT_sb = big.tile([128, KD, T], F32R, tag="xT")
            for kt in range(KD):
                nc.sync.dma_start(
                    out=xT_sb[:, kt, :], in_=xT[128 * kt : 128 * (kt + 1), :]
                )
            w_v_sb = big.tile([128, KD, DL], F32R, tag="wv")
            nc.sync.dma_start(
                out=w_v_sb, in_=w_v.rearrange("(kt p) n -> p kt n", p=128)
            )
            w_p_sb = big.tile([128, 2, D], F32R, tag="wp")
            nc.sync.dma_start(
                out=w_p_sb, in_=w_p.rearrange("(kt p) n -> p kt n", p=128)
            )
            b_qk_sb = big.tile([128, 4], F32, tag="bqk")
            nc.sync.dma_start(
                out=b_qk_sb, in_=b_qk.rearrange("(m p) -> p m", p=128)
            )
            ones4 = big.tile([128, 4], F32, tag="ones4")
            nc.vector.memset(ones4, 1.0)
            bv_ap = b_v[:]
            bv_bc = big.tile([128, DL], F32, tag="bv")
            nc.sync.dma_start(
                out=bv_bc,
                in_=bass.AP(tensor=bv_ap.tensor, offset=0, ap=[[0, 128], [1, DL]]),
            )
            # ---- diagonal causal masks (keep iff q >= k), offsets 128*i ----
            mask_f = big.tile([128, 512], F32, tag="maskf")
            masks = big.tile([128, 4, 512], F32R, tag="masks")
            for i in range(4):
                nc.gpsimd.memset(mask_f[:, :], 1.0)
                nc.gpsimd.affine_select(
                    out=mask_f[:, :],
                    in_=mask_f[:, :],
                    compare_op=mybir.AluOpType.is_ge,
                    fill=0.0,
                    base=-128 * i,
                    pattern=[[1, 512]],
                    channel_multiplier=-1,
                )
                nc.vector.tensor_copy(masks[:, i, :], mask_f[:, :])

            # ---- QKV projection --------------------------------------------
            qkT_sb = big.tile([128, 4, T], F32R, tag="qkT")
            for m in (0, 2, 1, 3):
                pq = [
                    ps.tile([128, 2, 512], F32, tag="pss", bufs=2, name=f"pq{m}{j}")
                    for j in range(2)
                ]
                for kt in range(KD):
                    wt = wqkp.tile([128, 128], F32R, tag="wqk")
                    nc.sync.dma_start(
                        out=wt,
                        in_=w_qk[128 * kt : 128 * (kt + 1), 128 * m : 128 * (m + 1)],
                    )
                    for n in range(NQ):
                        nc.tensor.matmul(
                            pq[n // 2][:, n % 2, :],
                            wt,
                            xT_sb[:, kt, 512 * n : 512 * (n + 1)],
                            start=(kt == 0),
                            stop=(kt == KD - 1),
                        )
                for j in range(2):
                    nc.scalar.activation(
                        qkT_sb[:, m, 1024 * j : 1024 * (j + 1)].rearrange(
                            "p (a b) -> p a b", a=2
                        ),
                        pq[j],
                        mybir.ActivationFunctionType.Identity,
                        bias=b_qk_sb[:, m : m + 1],
                    )
                if m >= 2:
                    nc.sync.dma_start(
                        out=kT_out[128 * (m - 2) : 128 * (m - 1), :],
                        in_=qkT_sb[:, m, :].bitcast(F32),
                    )
                # v in natural [t, dh] layout + ones column per head
                if m == 2:
                    v_sb = big.tile([128, NT, HPC * 65], F32R, tag="v")
                    for t in range(NT):
                        pv = ps.tile(
                            [128, 512], F32, tag="pv", bufs=3, name=f"psv{t}"
                        )
                        for kt in range(KD):
                            nc.tensor.matmul(
                                pv[:, :DL],
                                xT_sb[:, kt, 128 * t : 128 * (t + 1)],
                                w_v_sb[:, kt, :],
                                start=(kt == 0),
                                stop=(kt == KD - 1),
                            )
                        vslice = v_sb[:, t, :].rearrange("p (h c) -> p h c", h=HPC)
                        nc.vector.tensor_add(
                            vslice[:, :, 0:64],
                            pv[:, :DL].rearrange("p (h c) -> p h c", c=64),
                            bv_bc.rearrange("p (h c) -> p h c", c=64),
                        )
                        nc.vector.tensor_copy(
                            vslice[:, :, 64:65],
                            ones4.rearrange("p (h o) -> p h o", o=1),
                        )
                        nc.sync.dma_start(
                            out=v_out[128 * t : 128 * (t + 1), :].rearrange(
                                "p (h c) -> p h c", h=HPC
                            ),
                            in_=vslice[:, :, 0:64].bitcast(F32),
                        )

            # ---- attention -------------------------------------------------
            AT_sb = big.tile([128, 2, T], F32R, tag="AT")

            def emit_proj(qj):
                for t in range(4 * qj, 4 * (qj + 1)):
                    for dd in range(2):
                        po = ps.tile(
                            [128, 512], F32, tag="po", bufs=1, name=f"po{t}_{dd}"
                        )
                        for kt in range(2):
                            nc.tensor.matmul(
                                po,
                                AT_sb[:, kt, 128 * t : 128 * (t + 1)],
                                w_p_sb[:, kt, 512 * dd : 512 * (dd + 1)],
                                start=(kt == 0),
                                stop=(kt == 1),
                            )
                        o_sb = ostp.tile([128, 512], F32, tag="o")
                        nc.vector.tensor_copy(o_sb, po)
                        nc.sync.dma_start(
                            out=o_part[
                                128 * t : 128 * (t + 1), 512 * dd : 512 * (dd + 1)
                            ],
                            in_=o_sb,
                        )

            stages = [(hp, qj) for qj in range(NQ) for hp in range(2)]
            proj_done = set()
            for s, (hp, qj) in enumerate(stages):
                nk = 4 * qj + 4
                # proj for a query block one full stage after its norm
                rq = (s - 3) // 2
                if s >= 3 and s % 2 == 1 and rq >= 0 and rq not in proj_done:
                    proj_done.add(rq)
                    emit_proj(rq)
                pvs = [
                    ps.tile([65, 512], F32, tag="pv", bufs=3, name=f"pva{s}_{i}")
                    for i in range(2)
                ]
                exps = {}

                def s_step(kt, s=s, hp=hp, qj=qj, exps=exps):
                    pss = ps.tile(
                        [128, 2, 512], F32, tag="pss", bufs=2, name=f"pss{s}_{kt}"
                    )
                    for par in range(2):
                        p0 = 64 * par
                        nc.tensor.matmul(
                            pss[:, par, :],
                            qkT_sb[p0 : p0 + 64, 2 + hp, 128 * kt : 128 * (kt + 1)],
                            qkT_sb[p0 : p0 + 64, hp, 512 * qj : 512 * (qj + 1)],
                            start=True,
                            stop=True,
                        )
                    ep = expp.tile([128, 2, 512], F32R, tag="exp")
                    nc.scalar.activation(
                        ep, pss, mybir.ActivationFunctionType.Exp, scale=0.25
                    )
                    if kt >= 4 * qj:
                        for par in range(2):
                            nc.vector.tensor_mul(
                                ep[:, par, :], ep[:, par, :],
                                masks[:, kt - 4 * qj, :],
                            )
                    exps[kt] = ep

                def pv_step(kt, hp=hp, nk=nk, exps=exps, pvs=pvs):
                    for par in range(2):
                        h = 2 * hp + par
                        nc.tensor.matmul(
                            pvs[par],
                            v_sb[:, kt, 65 * h : 65 * (h + 1)],
                            exps[kt][:, par, :],
                            start=(kt == 0),
                            stop=(kt == nk - 1),
                        )
                    del exps[kt]

                for kt in range(nk):
                    s_step(kt)
                    if kt >= LOOK:
                        pv_step(kt - LOOK)
                for kt in range(max(0, nk - LOOK), nk):
                    pv_step(kt)

                # normalize: evacuate psum, reciprocal on a [64,8] reshape,
                # broadcast back over partitions, scale into AT
                for par in range(2):
                    un = nrmp.tile([65, 512], F32, tag="un")
                    nc.vector.tensor_copy(un, pvs[par])
                    d1 = nc.dram_tensor(f"nrm_d1_{s}_{par}", [64, 8], F32)
                    d2 = nc.dram_tensor(f"nrm_d2_{s}_{par}", [512], F32)
                    nc.sync.dma_start(
                        out=d1.rearrange("p c -> (p c)"), in_=un[64:65, :]
                    )
                    dn8 = nrmp.tile([64, 8], F32, tag="dn8")
                    nc.sync.dma_start(out=dn8, in_=d1[:, :])
                    rc8 = nrmp.tile([64, 8], F32, tag="rc8")
                    nc.vector.reciprocal(rc8, dn8)
                    nc.sync.dma_start(out=d2.rearrange("(p c) -> p c", p=64), in_=rc8)
                    rrow = nrmp.tile([1, 512], F32, tag="rrow")
                    nc.sync.dma_start(out=rrow, in_=d2[:])
                    bc = nrmp.tile([64, 512], F32, tag="bc")
                    nc.gpsimd.partition_broadcast(bc, rrow[0:1, :])
                    nc.vector.tensor_mul(
                        AT_sb[
                            64 * par : 64 * (par + 1), hp, 512 * qj : 512 * (qj + 1)
                        ],
                        un[0:64, :],
                        bc,
                    )
            for qj in range(NQ):
                if qj not in proj_done:
                    emit_proj(qj)
    nc.finalize()
    return nc


def _shard_inputs(x, w_attn, b_attn, w_proj):
    in_maps = []
    for c in range(8):
        bi, g = divmod(c, 4)
        cq = slice(DL * g, DL * (g + 1))
        ck = slice(D + DL * g, D + DL * (g + 1))
        cv = slice(2 * D + DL * g, 2 * D + DL * (g + 1))
        in_maps.append(
            {
                "xT": np.ascontiguousarray(x[bi].T),
                "w_qk": np.ascontiguousarray(
                    np.concatenate([w_attn[:, cq], w_attn[:, ck]], axis=1)
                ),
                "w_v": np.ascontiguousarray(w_attn[:, cv]),
                "b_qk": np.ascontiguousarray(
                    np.concatenate([b_attn[cq], b_attn[ck]])
                ),
                "b_v": np.ascontiguousarray(b_attn[cv]),
                "w_p": np.ascontiguousarray(w_proj[DL * g : DL * (g + 1), :]),
            }
        )
    return in_maps


def _gather(results, b_proj):
    a = np.zeros((B, T, D), np.float32)
    present = np.zeros((2, B, H, T, DH), np.float32)
    for c in range(8):
        bi, g = divmod(c, 4)
        r = results[c]
        a[bi] += r["o_part"]
        present[0, bi, HPC * g : HPC * (g + 1)] = (
            r["kT_out"].reshape(HPC, DH, T).transpose(0, 2, 1)
        )
        present[1, bi, HPC * g : HPC * (g + 1)] = (
            r["v_out"].reshape(T, HPC, DH).transpose(1, 0, 2)
        )
    a += b_proj.astype(np.float32)
    return a, present


def run(x, w_attn, b_attn, w_proj, b_proj, trace=False):
    if "nc" not in _CACHE:
        _CACHE["nc"] = build()
    in_maps = _shard_inputs(
        np.asarray(x, np.float32),
        np.asarray(w_attn, np.float32),
        np.asarray(b_attn, np.float32),
        np.asarray(w_proj, np.float32),
    )
    res = run_bass_kernel_spmd(
        _CACHE["nc"], in_maps, core_ids=list(range(8)), trace=trace
    )
    a, present = _gather(res.results, np.asarray(b_proj, np.float32))
    return (a, present), res


def kernel(x, w_attn, b_attn, w_proj, b_proj):
    (a, present), _ = run(x, w_attn, b_attn, w_proj, b_proj, trace=False)
    return a, present


# revision 8
# speedup vs baseline: 1.4176x; 1.1437x over previous
"""Causal multi-head attention block (QKV proj + attention + out proj) on 8
TRN2 NeuronCores.

Tensor-parallel sharding: core c handles batch b = c//4 and head group
g = c%4 (heads 4g..4g+3).  QKV weights are column-sharded, w_proj is
row-sharded; each core emits a partial output that the host sums per batch
(the row-parallel "all-reduce" happens on host since full outputs are
gathered anyway).

Shapes (hardcoded): B=2, T=2048, D=1024, H=16, dh=64.
Returns (a [B,T,D], present [2,B,H,T,dh]) matching the reference.

All matmuls run in float32r (TF32-like, full PE rate, ~1e-4 rel err).
Scores are computed transposed (S^T[k,q] = k @ q^T) so the attention
weights feed straight into the PV matmuls without transposing P: the
(v | ones) stationary trick makes row 64 of the PV psum the softmax
denominator for free.  Causality skips fully-masked k-tiles; the 4
diagonal-crossing tile shapes are masked with precomputed 0/1 tiles.
Head pairs share a 2-bank score psum so one Exp covers both heads.
"""
import numpy as np

import concourse.bass as bass
import concourse.bacc as bacc
import concourse.tile as tile
import concourse.mybir as mybir
from concourse.bass_utils import run_bass_kernel_spmd

B, T, D = 2, 2048, 1024
H = 16
DH = 64
HPC = 4            # heads per core
DL = HPC * DH      # 256 local head dims per core
KD = D // 128      # 8 contraction tiles over D
NT = T // 128      # 16 tiles over T (key positions / output rows)
NQ = T // 512      # 4 query tiles of 512
F32 = mybir.dt.float32
F32R = mybir.dt.float32r
LOOK = 2           # PV trails scores by this many k-tiles

_CACHE = {}


def build():
    nc = bacc.Bacc(
        "TRN2",
        target_bir_lowering=False,
        debug=False,
        enable_asserts=True,
        num_devices=8,
    )
    xT = nc.declare_dram_parameter("xT", (D, T), F32R, isOutput=False)
    w_qk = nc.declare_dram_parameter("w_qk", (D, 2 * DL), F32R, isOutput=False)
    w_v = nc.declare_dram_parameter("w_v", (D, DL), F32R, isOutput=False)
    b_qk = nc.declare_dram_parameter("b_qk", (2 * DL,), F32, isOutput=False)
    b_v = nc.declare_dram_parameter("b_v", (DL,), F32, isOutput=False)
    w_p = nc.declare_dram_parameter("w_p", (DL, D), F32R, isOutput=False)
    o_part = nc.declare_dram_parameter("o_part", (T, D), F32, isOutput=True)
    kT_out = nc.declare_dram_parameter("kT_out", (DL, T), F32, isOutput=True)
    v_out = nc.declare_dram_parameter("v_out", (T, DL), F32, isOutput=True)

    with tile.TileContext(nc) as tc:
        with (
            tc.tile_pool(name="big", bufs=1) as big,
            tc.tile_pool(name="wqk", bufs=4) as wqkp,
            tc.tile_pool(name="exp", bufs=4) as expp,
            tc.tile_pool(name="nrm", bufs=3) as nrmp,
            tc.tile_pool(name="ost", bufs=2) as ostp,
            tc.tile_pool(name="ps", bufs=1, space="PSUM") as ps,
        ):
            # ---- resident inputs (all via fast HW-DGE DMA) -----------------
            xT_sb = big.tile([128, KD, T], F32R, tag="xT")
            for kt in range(KD):
                for n in range(NQ):
                    eng = nc.sync if (kt * NQ + n) % 2 == 0 else nc.gpsimd
                    eng.dma_start(
                        out=xT_sb[:, kt, 512 * n : 512 * (n + 1)],
                        in_=xT[128 * kt : 128 * (kt + 1), 512 * n : 512 * (n + 1)],
                    )
            w_v_sb = big.tile([128, KD, DL], F32R, tag="wv")
            nc.sync.dma_start(
                out=w_v_sb, in_=w_v.rearrange("(kt p) n -> p kt n", p=128)
            )
            w_p_sb = big.tile([128, 2, D], F32R, tag="wp")
            nc.sync.dma_start(
                out=w_p_sb, in_=w_p.rearrange("(kt p) n -> p kt n", p=128)
            )
            b_qk_sb = big.tile([128, 4], F32, tag="bqk")
            nc.sync.dma_start(
                out=b_qk_sb, in_=b_qk.rearrange("(m p) -> p m", p=128)
            )
            ones4 = big.tile([128, 4], F32, tag="ones4")
            nc.vector.memset(ones4, 1.0)
            bv_ap = b_v[:]
            bv_bc = big.tile([128, DL], F32, tag="bv")
            nc.sync.dma_start(
                out=bv_bc,
                in_=bass.AP(tensor=bv_ap.tensor, offset=0, ap=[[0, 128], [1, DL]]),
            )
            # ---- diagonal causal masks (keep iff q >= k), offsets 128*i ----
            mask_f = big.tile([128, 512], F32, tag="maskf")
            masks = big.tile([128, 4, 512], F32R, tag="masks")
            for i in range(4):
                nc.gpsimd.memset(mask_f[:, :], 1.0)
                nc.gpsimd.affine_select(
                    out=mask_f[:, :],
                    in_=mask_f[:, :],
                    compare_op=mybir.AluOpType.is_ge,
                    fill=0.0,
                    base=-128 * i,
                    pattern=[[1, 512]],
                    channel_multiplier=-1,
                )
                nc.vector.tensor_copy(masks[:, i, :], mask_f[:, :])

            # ---- QKV projection --------------------------------------------
            qkT_sb = big.tile([128, 4, T], F32R, tag="qkT")
            for m in (0, 2, 1, 3):
                pq = [
                    ps.tile([128, 2, 512], F32, tag="pss", bufs=2, name=f"pq{m}{j}")
                    for j in range(2)
                ]
                for kt in range(KD):
                    wt = wqkp.tile([128, 128], F32R, tag="wqk")
                    nc.sync.dma_start(
                        out=wt,
                        in_=w_qk[128 * kt : 128 * (kt + 1), 128 * m : 128 * (m + 1)],
                    )
                    for n in range(NQ):
                        nc.tensor.matmul(
                            pq[n // 2][:, n % 2, :],
                            wt,
                            xT_sb[:, kt, 512 * n : 512 * (n + 1)],
                            start=(kt == 0),
                            stop=(kt == KD - 1),
                        )
                for j in range(2):
                    nc.scalar.activation(
                        qkT_sb[:, m, 1024 * j : 1024 * (j + 1)].rearrange(
                            "p (a b) -> p a b", a=2
                        ),
                        pq[j],
                        mybir.ActivationFunctionType.Identity,
                        bias=b_qk_sb[:, m : m + 1],
                    )
                if m >= 2:
                    nc.sync.dma_start(
                        out=kT_out[128 * (m - 2) : 128 * (m - 1), :],
                        in_=qkT_sb[:, m, :].bitcast(F32),
                    )
                # v in natural [t, dh] layout + ones column per head
                if m == 2:
                    v_sb = big.tile([128, NT, HPC * 65], F32R, tag="v")
                    for t in range(NT):
                        pv = ps.tile(
                            [128, 512], F32, tag="pv", bufs=3, name=f"psv{t}"
                        )
                        for kt in range(KD):
                            nc.tensor.matmul(
                                pv[:, :DL],
                                xT_sb[:, kt, 128 * t : 128 * (t + 1)],
                                w_v_sb[:, kt, :],
                                start=(kt == 0),
                                stop=(kt == KD - 1),
                            )
                        vslice = v_sb[:, t, :].rearrange("p (h c) -> p h c", h=HPC)
                        nc.vector.tensor_add(
                            vslice[:, :, 0:64],
                            pv[:, :DL].rearrange("p (h c) -> p h c", c=64),
                            bv_bc.rearrange("p (h c) -> p h c", c=64),
                        )
                        nc.vector.tensor_copy(
                            vslice[:, :, 64:65],
                            ones4.rearrange("p (h o) -> p h o", o=1),
                        )
                        nc.sync.dma_start(
                            out=v_out[128 * t : 128 * (t + 1), :].rearrange(
                                "p (h c) -> p h c", h=HPC
                            ),
                            in_=vslice[:, :, 0:64].bitcast(F32),
                        )

            # ---- attention -------------------------------------------------
            AT_sb = big.tile([128, 2, T], F32R, tag="AT")

            def emit_proj(qj):
                for t in range(4 * qj, 4 * (qj + 1)):
                    for dd in range(2):
                        po = ps.tile(
                            [128, 512], F32, tag="po", bufs=1, name=f"po{t}_{dd}"
                        )
                        for kt in range(2):
                            nc.tensor.matmul(
                                po,
                                AT_sb[:, kt, 128 * t : 128 * (t + 1)],
                                w_p_sb[:, kt, 512 * dd : 512 * (dd + 1)],
                                start=(kt == 0),
                                stop=(kt == 1),
                            )
                        o_sb = ostp.tile([128, 512], F32, tag="o")
                        nc.vector.tensor_copy(o_sb, po)
                        nc.sync.dma_start(
                            out=o_part[
                                128 * t : 128 * (t + 1), 512 * dd : 512 * (dd + 1)
                            ],
                            in_=o_sb,
                        )

            stages = [(hp, qj) for qj in reversed(range(NQ)) for hp in range(2)]
            proj_done = set()
            for s, (hp, qj) in enumerate(stages):
                nk = 4 * qj + 4
                # proj for a query block one full stage after its norm
                if s >= 3 and s % 2 == 1:
                    rq = stages[s - 2][1]
                    if rq not in proj_done:
                        proj_done.add(rq)
                        emit_proj(rq)
                pvs = [
                    ps.tile([65, 512], F32, tag="pv", bufs=3, name=f"pva{s}_{i}")
                    for i in range(2)
                ]
                exps = {}

                def s_step(kt, s=s, hp=hp, qj=qj, exps=exps):
                    pss = ps.tile(
                        [128, 2, 512], F32, tag="pss", bufs=2, name=f"pss{s}_{kt}"
                    )
                    for par in range(2):
                        p0 = 64 * par
                        nc.tensor.matmul(
                            pss[:, par, :],
                            qkT_sb[p0 : p0 + 64, 2 + hp, 128 * kt : 128 * (kt + 1)],
                            qkT_sb[p0 : p0 + 64, hp, 512 * qj : 512 * (qj + 1)],
                            start=True,
                            stop=True,
                        )
                    ep = expp.tile([128, 2, 512], F32R, tag="exp")
                    nc.scalar.activation(
                        ep, pss, mybir.ActivationFunctionType.Exp, scale=0.25
                    )
                    if kt >= 4 * qj:
                        for par in range(2):
                            nc.vector.tensor_mul(
                                ep[:, par, :], ep[:, par, :],
                                masks[:, kt - 4 * qj, :],
                            )
                    exps[kt] = ep

                def pv_step(kt, hp=hp, nk=nk, exps=exps, pvs=pvs):
                    for par in range(2):
                        h = 2 * hp + par
                        nc.tensor.matmul(
                            pvs[par],
                            v_sb[:, kt, 65 * h : 65 * (h + 1)],
                            exps[kt][:, par, :],
                            start=(kt == 0),
                            stop=(kt == nk - 1),
                        )
                    del exps[kt]

                for kt in range(nk):
                    s_step(kt)
                    if kt >= LOOK:
                        pv_step(kt - LOOK)
                for kt in range(max(0, nk - LOOK), nk):
                    pv_step(kt)

                # normalize: evacuate psums, reshape both denom rows to
                # [128,8] via a DRAM bounce, one reciprocal, broadcast the
                # reciprocals straight from DRAM, scale into AT
                d1 = nc.dram_tensor(f"nrm_d1_{s}", [2, 512], F32)
                d2 = nc.dram_tensor(f"nrm_d2_{s}", [2, 512], F32)
                uns = []
                for par in range(2):
                    un = nrmp.tile([65, 512], F32, tag="un")
                    nc.vector.tensor_copy(un, pvs[par])
                    uns.append(un)
                    nc.sync.dma_start(out=d1[par, :], in_=un[64:65, :])
                dn8 = nrmp.tile([128, 8], F32, tag="dn8")
                nc.sync.dma_start(
                    out=dn8, in_=d1.rearrange("a (p c) -> (a p) c", p=64)
                )
                rc8 = nrmp.tile([128, 8], F32, tag="rc8")
                nc.vector.reciprocal(rc8, dn8)
                nc.gpsimd.dma_start(
                    out=d2.rearrange("a (p c) -> (a p) c", p=64), in_=rc8
                )
                for par in range(2):
                    bc = nrmp.tile([64, 512], F32, tag="bc")
                    nc.gpsimd.dma_start(
                        out=bc,
                        in_=bass.AP(
                            tensor=d2[:].tensor,
                            offset=512 * par,
                            ap=[[0, 64], [1, 512]],
                        ),
                    )
                    nc.vector.tensor_mul(
                        AT_sb[
                            64 * par : 64 * (par + 1), hp, 512 * qj : 512 * (qj + 1)
                        ],
                        uns[par][0:64, :],
                        bc,
                    )
            for qj in range(NQ):
                if qj not in proj_done:
                    emit_proj(qj)
    nc.finalize()
    return nc


def _shard_inputs(x, w_attn, b_attn, w_proj):
    in_maps = []
    for c in range(8):
        bi, g = divmod(c, 4)
        cq = slice(DL * g, DL * (g + 1))
        ck = slice(D + DL * g, D + DL * (g + 1))
        cv = slice(2 * D + DL * g, 2 * D + DL * (g + 1))
        in_maps.append(
            {
                "xT": np.ascontiguousarray(x[bi].T),
                "w_qk": np.ascontiguousarray(
                    np.concatenate([w_attn[:, cq], w_attn[:, ck]], axis=1)
                ),
                "w_v": np.ascontiguousarray(w_attn[:, cv]),
                "b_qk": np.ascontiguousarray(
                    np.concatenate([b_attn[cq], b_attn[ck]])
                ),
                "b_v": np.ascontiguousarray(b_attn[cv]),
                "w_p": np.ascontiguousarray(w_proj[DL * g : DL * (g + 1), :]),
            }
        )
    return in_maps


def _gather(results, b_proj):
    a = np.zeros((B, T, D), np.float32)
    present = np.zeros((2, B, H, T, DH), np.float32)
    for c in range(8):
        bi, g = divmod(c, 4)
        r = results[c]
        a[bi] += r["o_part"]
        present[0, bi, HPC * g : HPC * (g + 1)] = (
            r["kT_out"].reshape(HPC, DH, T).transpose(0, 2, 1)
        )
        present[1, bi, HPC * g : HPC * (g + 1)] = (
            r["v_out"].reshape(T, HPC, DH).transpose(1, 0, 2)
        )
    a += b_proj.astype(np.float32)
    return a, present


def run(x, w_attn, b_attn, w_proj, b_proj, trace=False):
    if "nc" not in _CACHE:
        _CACHE["nc"] = build()
    in_maps = _shard_inputs(
        np.asarray(x, np.float32),
        np.asarray(w_attn, np.float32),
        np.asarray(b_attn, np.float32),
        np.asarray(w_proj, np.float32),
    )
    res = run_bass_kernel_spmd(
        _CACHE["nc"], in_maps, core_ids=list(range(8)), trace=trace
    )
    a, present = _gather(res.results, np.asarray(b_proj, np.float32))
    return (a, present), res


def kernel(x, w_attn, b_attn, w_proj, b_proj):
    (a, present), _ = run(x, w_attn, b_attn, w_proj, b_proj, trace=False)
    return a, present
